# revision 15
# baseline (speedup 1.0000x reference)
"""Trainium2 Bass kernel for 16-head attention (B=4, S=2048, D=1024).

Sharding: 8 cores = 4 batches x 2 head-groups. Core c handles batch c//2,
heads (c%2)*8 .. +8. Each core computes a partial projection output
[S, D]; the host sums the two head-group partials per batch and adds
b_proj. No collectives.

Per-core layout trick: host feeds x[b] transposed (xT [D, S]), so the QKV
matmuls produce Q^T / K^T in [qkv-col, seq] layout directly, scores are
computed transposed ([sk, sq]) and softmax is done without max-subtraction
(inputs are bounded; exp stays well inside fp32/bf16 range). V is
ones-augmented so the attn@V matmul also yields softmax row-sums for free;
normalization uses a DVE reciprocal + a K=1 outer-product matmul to
broadcast the per-column scale across partitions.
"""

import sys
import os

sys.path.insert(0, "/opt/trn_rl_repo")

import numpy as np
import ml_dtypes

BF = ml_dtypes.bfloat16

DIM = 1024
N_HEADS = 16
HD = 64
B = 4
S = 2048
HPC = 8          # heads per core
GC = HPC * HD    # 512 columns per head-group
N_CORES = 8
SCALE = HD ** -0.5

_CACHE = {}


def _build_bass():
    import concourse.bass as bass
    import concourse.mybir as mybir
    import concourse.tile as tile
    from concourse import bacc

    f32 = mybir.dt.float32
    bf16 = mybir.dt.bfloat16
    EXP = mybir.ActivationFunctionType.Exp

    nc = bacc.Bacc("TRN2", target_bir_lowering=False, debug=False,
                   num_devices=N_CORES)

    xT = nc.dram_tensor("xT", [DIM, S], bf16, kind="ExternalInput").ap()
    wq = nc.dram_tensor("wq", [DIM, GC], bf16, kind="ExternalInput").ap()
    wk = nc.dram_tensor("wk", [DIM, GC], bf16, kind="ExternalInput").ap()
    wv = nc.dram_tensor("wv", [DIM, GC], bf16, kind="ExternalInput").ap()
    wp = nc.dram_tensor("wp", [GC, DIM], bf16, kind="ExternalInput").ap()
    # q/k biases pre-broadcast on host: [128, m-tile*512] per-partition value
    bq = nc.dram_tensor("bq", [128, 2048], f32, kind="ExternalInput").ap()
    bk = nc.dram_tensor("bk", [128, 2048], f32, kind="ExternalInput").ap()
    bvb = nc.dram_tensor("bvb", [128, GC], f32, kind="ExternalInput").ap()
    out = nc.dram_tensor("out", [S, DIM], f32, kind="ExternalOutput").ap()

    KD = DIM // 128   # 8 k-tiles over D
    NQ = GC // 128    # 4 tiles over the 512 head-group columns
    NS = S // 512     # 4 seq chunks of 512
    ST = S // 128     # 16 seq tiles of 128

    with tile.TileContext(nc) as tc:
        with tc.tile_pool(name="const", bufs=1) as cp:
            xTs = []
            for k in range(KD):
                t = cp.tile([128, S], bf16, name=f"xTs{k}")
                nc.sync.dma_start(t[:], xT[k * 128:(k + 1) * 128, :])
                xTs.append(t)
            wqs, wks, wvs = [], [], []
            for k in range(KD):
                for lst, src, nm in ((wqs, wq, "q"), (wks, wk, "k"),
                                     (wvs, wv, "v")):
                    t = cp.tile([128, GC], bf16, name=f"w{nm}s{k}")
                    nc.sync.dma_start(t[:], src[k * 128:(k + 1) * 128, :])
                    lst.append(t)
            wps = []
            for k in range(NQ):
                t = cp.tile([128, DIM], bf16, name=f"wps{k}")
                nc.sync.dma_start(t[:], wp[k * 128:(k + 1) * 128, :])
                wps.append(t)
            bq_sb = cp.tile([128, 2048], f32, name="bq_sb")
            nc.sync.dma_start(bq_sb[:], bq[:, :])
            bk_sb = cp.tile([128, 2048], f32, name="bk_sb")
            nc.sync.dma_start(bk_sb[:], bk[:, :])
            bvb_sb = cp.tile([128, GC], f32, name="bvb_sb")
            nc.sync.dma_start(bvb_sb[:], bvb[:, :])
            ones_sb = cp.tile([128, 64], bf16, name="ones_sb")
            nc.any.memset(ones_sb[:], 1.0)

            QT = [cp.tile([128, S], bf16, name=f"QT{m}") for m in range(NQ)]
            KT = [cp.tile([128, S], bf16, name=f"KT{m}") for m in range(NQ)]
            # V tiles: per head 65 cols (64 data + trailing ones column)
            Vt = [cp.tile([128, HPC * 65], bf16, name=f"Vt{s}")
                  for s in range(ST)]
            OT = [cp.tile([128, S], bf16, name=f"OT{m}") for m in range(NQ)]

            for s in range(ST):
                ones_cols = Vt[s][:, :].rearrange(
                    "p (h c) -> p h c", c=65)[:, :, 64:65]
                nc.any.memset(ones_cols, 1.0)

            # ---- QKV projections ----
            with tc.tile_pool(name="psq", bufs=2, space="PSUM") as psq:
                for dst, ws, bias in ((QT, wqs, bq_sb), (KT, wks, bk_sb)):
                    for m in range(NQ):
                        for n in range(NS):
                            ps = psq.tile([128, 512], f32, tag="ps",
                                          name=f"psqkv{m}{n}")
                            for k in range(KD):
                                nc.tensor.matmul(
                                    ps[:],
                                    lhsT=ws[k][:, m * 128:(m + 1) * 128],
                                    rhs=xTs[k][:, n * 512:(n + 1) * 512],
                                    start=(k == 0), stop=(k == KD - 1))
                            nc.vector.tensor_add(
                                dst[m][:, n * 512:(n + 1) * 512], ps[:],
                                bias[:, m * 512:(m + 1) * 512])
                for s in range(ST):
                    ps = psq.tile([128, 512], f32, tag="ps", name=f"psv{s}")
                    for k in range(KD):
                        nc.tensor.matmul(
                            ps[:],
                            lhsT=xTs[k][:, s * 128:(s + 1) * 128],
                            rhs=wvs[k][:, :],
                            start=(k == 0), stop=(k == KD - 1))
                    src3 = ps[:].rearrange("p (h c) -> p h c", c=64)
                    bv3 = bvb_sb[:].rearrange("p (h c) -> p h c", c=64)
                    dst3 = Vt[s][:, :].rearrange(
                        "p (h c) -> p h c", c=65)[:, :, 0:64]
                    nc.vector.tensor_add(dst3, src3, bv3)

            # ---- attention (per head-pair, per sq chunk) ----
            with tc.tile_pool(name="psS", bufs=2, space="PSUM") as psSp, \
                 tc.tile_pool(name="psO", bufs=4, space="PSUM") as psOp, \
                 tc.tile_pool(name="pbuf", bufs=4) as pbufp, \
                 tc.tile_pool(name="misc", bufs=4) as miscp:
                for hp in range(NQ):
                    for n in range(NS):
                        sq = slice(n * 512, (n + 1) * 512)
                        oA = psOp.tile([128, 512], f32, tag="o",
                                       name=f"oA{hp}{n}")
                        oB = psOp.tile([128, 512], f32, tag="o",
                                       name=f"oB{hp}{n}")
                        for j in range(ST):
                            sk = slice(j * 128, (j + 1) * 128)
                            for half, oPS in ((0, oA), (1, oB)):
                                pr = slice(half * 64, half * 64 + 64)
                                sS = psSp.tile([128, 512], f32, tag="s",
                                               name=f"sS{hp}{n}{j}{half}")
                                nc.tensor.matmul(
                                    sS[:], lhsT=KT[hp][pr, sk],
                                    rhs=QT[hp][pr, sq],
                                    start=True, stop=True)
                                pT = pbufp.tile([128, 512], bf16, tag="p",
                                                name=f"pT{hp}{n}{j}{half}")
                                nc.scalar.activation(pT[:], sS[:], EXP,
                                                     scale=SCALE)
                                h = hp * 2 + half
                                nc.tensor.matmul(
                                    oPS[0:65, :],
                                    lhsT=Vt[j][:, h * 65:h * 65 + 65],
                                    rhs=pT[:],
                                    start=(j == 0), stop=(j == ST - 1))
                        # normalize: r = 1/rowsum; gpsimd partition-broadcast
                        for half, oPS in ((0, oA), (1, oB)):
                            r = miscp.tile([128, 512], bf16, tag="r",
                                           name=f"r{hp}{n}{half}")
                            with nc.allow_low_precision(
                                    reason="bf16 softmax denom, matches "
                                           "bf16 matmul precision"):
                                nc.vector.reciprocal(r[64:65, :],
                                                     oPS[64:65, :])
                            pb = psSp.tile([128, 512], f32, tag="s",
                                           name=f"pb{hp}{n}{half}")
                            nc.tensor.matmul(pb[0:64, :],
                                             lhsT=ones_sb[64:65, 0:64],
                                             rhs=r[64:65, :],
                                             start=True, stop=True)
                            rb = miscp.tile([64, 512], bf16, tag="rb",
                                            name=f"rb{hp}{n}{half}")
                            nc.vector.tensor_copy(rb[:, :], pb[0:64, :])
                            if half == 0:
                                nc.vector.tensor_mul(
                                    OT[hp][0:64, sq], oPS[0:64, :],
                                    rb[:, :])
                            else:
                                stB = miscp.tile([64, 512], bf16, tag="st",
                                                 name=f"stB{hp}{n}")
                                nc.vector.tensor_mul(stB[:], oPS[0:64, :],
                                                     rb[:, :])
                                nc.sync.dma_start(OT[hp][64:128, sq], stB[:])

            # ---- output projection (partial over this head-group) ----
            with tc.tile_pool(name="psD", bufs=2, space="PSUM") as psDp, \
                 tc.tile_pool(name="ost", bufs=3) as ostp:
                for m in range(ST):
                    for nn in range(2):
                        ps = psDp.tile([128, 512], f32, tag="d",
                                       name=f"psd{m}{nn}")
                        for k in range(NQ):
                            nc.tensor.matmul(
                                ps[:],
                                lhsT=OT[k][:, m * 128:(m + 1) * 128],
                                rhs=wps[k][:, nn * 512:(nn + 1) * 512],
                                start=(k == 0), stop=(k == NQ - 1))
                        ob = ostp.tile([128, 512], f32, tag="ob",
                                       name=f"ob{m}{nn}")
                        nc.vector.tensor_copy(ob[:], ps[:])
                        nc.sync.dma_start(
                            out[m * 128:(m + 1) * 128,
                                nn * 512:(nn + 1) * 512], ob[:])
    nc.compile()
    return nc


def _get_nc():
    if "nc" not in _CACHE:
        _CACHE["nc"] = _build_bass()
    return _CACHE["nc"]


def _in_maps(x, w_qkv, b_qkv, w_proj, b_proj):
    x = np.asarray(x, np.float32)
    w_qkv = np.asarray(w_qkv, np.float32)
    b_qkv = np.asarray(b_qkv, np.float32)
    w_proj = np.asarray(w_proj, np.float32)
    maps = []
    for c in range(N_CORES):
        b, g = divmod(c, 2)
        cols = slice(g * GC, (g + 1) * GC)
        wqs = w_qkv[:, 0 * DIM:1 * DIM][:, cols]
        wks = w_qkv[:, 1 * DIM:2 * DIM][:, cols]
        wvs = w_qkv[:, 2 * DIM:3 * DIM][:, cols]
        bqs = b_qkv[0 * DIM:1 * DIM][cols]
        bks = b_qkv[1 * DIM:2 * DIM][cols]
        bvs = b_qkv[2 * DIM:3 * DIM][cols]
        rows = slice(g * GC, (g + 1) * GC)
        maps.append({
            "xT": np.ascontiguousarray(x[b].T).astype(BF),
            "wq": wqs.astype(BF),
            "wk": wks.astype(BF),
            "wv": wvs.astype(BF),
            "wp": w_proj[rows, :].astype(BF),
            "bq": np.ascontiguousarray(
                np.broadcast_to(bqs.reshape(4, 128).T[:, :, None],
                                (128, 4, 512)).reshape(128, 2048)),
            "bk": np.ascontiguousarray(
                np.broadcast_to(bks.reshape(4, 128).T[:, :, None],
                                (128, 4, 512)).reshape(128, 2048)),
            "bvb": np.broadcast_to(bvs, (128, GC)).copy(),
        })
    return maps


def kernel(x, w_qkv, b_qkv, w_proj, b_proj, _trace=False):
    from concourse import bass_utils
    nc = _get_nc()
    maps = _in_maps(x, w_qkv, b_qkv, w_proj, b_proj)
    res = bass_utils.run_bass_kernel_spmd(nc, maps,
                                          core_ids=list(range(N_CORES)),
                                          trace=_trace)
    _CACHE["last_result"] = res
    b_proj = np.asarray(b_proj, np.float32)
    outs = np.empty((B, S, DIM), np.float32)
    for b in range(B):
        outs[b] = (res.results[2 * b]["out"] + res.results[2 * b + 1]["out"]
                   + b_proj)
    return outs


# revision 18
# speedup vs baseline: 1.4408x; 1.4408x over previous
"""Trainium2 Bass kernel for 16-head attention (B=4, S=2048, D=1024).

Sharding: 8 cores = 4 batches x 2 head-groups. Core c handles batch c//2,
heads (c%2)*8 .. +8. Each core computes a partial projection output
[S, D]; the host sums the two head-group partials per batch and adds
b_proj. No collectives.

Per-core layout trick: host feeds x[b] transposed (xT [D, S]), so the QKV
matmuls produce Q^T / K^T in [qkv-col, seq] layout directly, scores are
computed transposed ([sk, sq]) and softmax is done without max-subtraction
(inputs are bounded; exp stays well inside fp32/bf16 range). V is
ones-augmented so the attn@V matmul also yields softmax row-sums for free;
normalization uses a DVE reciprocal + a K=1 outer-product matmul to
broadcast the per-column scale across partitions.
"""

import sys
import os

sys.path.insert(0, "/opt/trn_rl_repo")

import numpy as np
import ml_dtypes

BF = ml_dtypes.bfloat16

DIM = 1024
N_HEADS = 16
HD = 64
B = 4
S = 2048
HPC = 8          # heads per core
GC = HPC * HD    # 512 columns per head-group
N_CORES = 8
SCALE = HD ** -0.5

_CACHE = {}


def _build_bass():
    import concourse.bass as bass
    import concourse.mybir as mybir
    import concourse.tile as tile
    from concourse import bacc

    f32 = mybir.dt.float32
    bf16 = mybir.dt.bfloat16
    EXP = mybir.ActivationFunctionType.Exp

    nc = bacc.Bacc("TRN2", target_bir_lowering=False, debug=False,
                   num_devices=N_CORES)

    xT = nc.dram_tensor("xT", [DIM, S], bf16, kind="ExternalInput").ap()
    wq = nc.dram_tensor("wq", [DIM, GC], bf16, kind="ExternalInput").ap()
    wk = nc.dram_tensor("wk", [DIM, GC], bf16, kind="ExternalInput").ap()
    wv = nc.dram_tensor("wv", [DIM, GC], bf16, kind="ExternalInput").ap()
    wp = nc.dram_tensor("wp", [GC, DIM], bf16, kind="ExternalInput").ap()
    # q/k biases pre-broadcast on host: [128, m-tile*512] per-partition value
    bq = nc.dram_tensor("bq", [128, 2048], f32, kind="ExternalInput").ap()
    bk = nc.dram_tensor("bk", [128, 2048], f32, kind="ExternalInput").ap()
    bvb = nc.dram_tensor("bvb", [128, GC], f32, kind="ExternalInput").ap()
    out = nc.dram_tensor("out", [S, DIM], f32, kind="ExternalOutput").ap()

    KD = DIM // 128   # 8 k-tiles over D
    NQ = GC // 128    # 4 tiles over the 512 head-group columns
    NS = S // 512     # 4 seq chunks of 512
    ST = S // 128     # 16 seq tiles of 128

    with tile.TileContext(nc) as tc:
        with tc.tile_pool(name="const", bufs=1) as cp:
            xTs = []
            for k in range(KD):
                t = cp.tile([128, S], bf16, name=f"xTs{k}")
                nc.sync.dma_start(t[:], xT[k * 128:(k + 1) * 128, :])
                xTs.append(t)
            wqs, wks, wvs = [], [], []
            for k in range(KD):
                for lst, src, nm in ((wqs, wq, "q"), (wks, wk, "k"),
                                     (wvs, wv, "v")):
                    t = cp.tile([128, GC], bf16, name=f"w{nm}s{k}")
                    nc.sync.dma_start(t[:], src[k * 128:(k + 1) * 128, :])
                    lst.append(t)
            wps = []
            for k in range(NQ):
                t = cp.tile([128, DIM], bf16, name=f"wps{k}")
                nc.sync.dma_start(t[:], wp[k * 128:(k + 1) * 128, :])
                wps.append(t)
            bq_sb = cp.tile([128, 2048], f32, name="bq_sb")
            nc.sync.dma_start(bq_sb[:], bq[:, :])
            bk_sb = cp.tile([128, 2048], f32, name="bk_sb")
            nc.sync.dma_start(bk_sb[:], bk[:, :])
            bvb_sb = cp.tile([128, GC], f32, name="bvb_sb")
            nc.sync.dma_start(bvb_sb[:], bvb[:, :])
            ones_sb = cp.tile([128, 64], f32, name="ones_sb")
            nc.any.memset(ones_sb[:], 1.0)

            QT = [cp.tile([128, S], bf16, name=f"QT{m}") for m in range(NQ)]
            KT = [cp.tile([128, S], bf16, name=f"KT{m}") for m in range(NQ)]
            # V tiles: per head 65 cols (64 data + trailing ones column)
            Vt = [cp.tile([128, HPC * 65], bf16, name=f"Vt{s}")
                  for s in range(ST)]
            OT = [cp.tile([128, S], bf16, name=f"OT{m}") for m in range(NQ)]

            for s in range(ST):
                ones_cols = Vt[s][:, :].rearrange(
                    "p (h c) -> p h c", c=65)[:, :, 64:65]
                nc.any.memset(ones_cols, 1.0)

            # ---- QKV projections ----
            with tc.tile_pool(name="psq", bufs=2, space="PSUM") as psq:
                for dst, ws, bias in ((QT, wqs, bq_sb), (KT, wks, bk_sb)):
                    for m in range(NQ):
                        for n in range(NS):
                            ps = psq.tile([128, 512], f32, tag="ps",
                                          name=f"psqkv{m}{n}")
                            for k in range(KD):
                                nc.tensor.matmul(
                                    ps[:],
                                    lhsT=ws[k][:, m * 128:(m + 1) * 128],
                                    rhs=xTs[k][:, n * 512:(n + 1) * 512],
                                    start=(k == 0), stop=(k == KD - 1))
                            nc.vector.tensor_add(
                                dst[m][:, n * 512:(n + 1) * 512], ps[:],
                                bias[:, m * 512:(m + 1) * 512])
                for s in range(ST):
                    ps = psq.tile([128, 512], f32, tag="ps", name=f"psv{s}")
                    for k in range(KD):
                        nc.tensor.matmul(
                            ps[:],
                            lhsT=xTs[k][:, s * 128:(s + 1) * 128],
                            rhs=wvs[k][:, :],
                            start=(k == 0), stop=(k == KD - 1))
                    src3 = ps[:].rearrange("p (h c) -> p h c", c=64)
                    bv3 = bvb_sb[:].rearrange("p (h c) -> p h c", c=64)
                    dst3 = Vt[s][:, :].rearrange(
                        "p (h c) -> p h c", c=65)[:, :, 0:64]
                    nc.vector.tensor_add(dst3, src3, bv3)

            # ---- attention (per sq chunk, per head-pair) ----
            with tc.tile_pool(name="psS", bufs=2, space="PSUM") as psSp, \
                 tc.tile_pool(name="psO", bufs=3, space="PSUM") as psOp, \
                 tc.tile_pool(name="psB", bufs=1, space="PSUM") as psBp, \
                 tc.tile_pool(name="pbuf", bufs=3) as pbufp, \
                 tc.tile_pool(name="misc", bufs=4) as miscp:
                for n in range(NS):
                    sq = slice(n * 512, (n + 1) * 512)
                    for hp in range(NQ):
                        oA = psOp.tile([128, 512], f32, tag="o",
                                       name=f"oA{hp}{n}")
                        oB = psOp.tile([128, 512], f32, tag="o",
                                       name=f"oB{hp}{n}")
                        for j in range(ST):
                            sk = slice(j * 128, (j + 1) * 128)
                            # both heads' scores into one 2-bank psum tile;
                            # the two K=64 matmuls row-tile and overlap in PE
                            sS = psSp.tile([128, 1024], f32, tag="s",
                                           name=f"sS{hp}{n}{j}")
                            nc.tensor.matmul(
                                sS[:, 0:512], lhsT=KT[hp][0:64, sk],
                                rhs=QT[hp][0:64, sq],
                                start=True, stop=True)
                            nc.tensor.matmul(
                                sS[:, 512:1024], lhsT=KT[hp][64:128, sk],
                                rhs=QT[hp][64:128, sq],
                                start=True, stop=True)
                            pT = pbufp.tile([128, 1024], bf16, tag="p",
                                            name=f"pT{hp}{n}{j}")
                            nc.scalar.activation(pT[:], sS[:], EXP,
                                                 scale=SCALE)
                            ha = hp * 2
                            nc.tensor.matmul(
                                oA[0:65, :],
                                lhsT=Vt[j][:, ha * 65:ha * 65 + 65],
                                rhs=pT[:, 0:512],
                                start=(j == 0), stop=(j == ST - 1))
                            nc.tensor.matmul(
                                oB[0:65, :],
                                lhsT=Vt[j][:, ha * 65 + 65:ha * 65 + 130],
                                rhs=pT[:, 512:1024],
                                start=(j == 0), stop=(j == ST - 1))
                        # normalize: approx-recip of rowsum row, broadcast
                        # across partitions via K=1 fp32 outer-product
                        for half, oPS in ((0, oA), (1, oB)):
                            r = miscp.tile([128, 512], f32, tag="r",
                                           name=f"r{hp}{n}{half}")
                            nc.vector.reciprocal(r[64:65, :],
                                                 oPS[64:65, :])
                            pb = psBp.tile([128, 512], f32, tag="b",
                                           name=f"pb{hp}{n}{half}")
                            nc.tensor.matmul(pb[0:64, :],
                                             lhsT=ones_sb[64:65, 0:64],
                                             rhs=r[64:65, :],
                                             start=True, stop=True)
                            rb = miscp.tile([64, 512], bf16, tag="rb",
                                            name=f"rb{hp}{n}{half}")
                            nc.vector.tensor_copy(rb[:, :], pb[0:64, :])
                            if half == 0:
                                nc.vector.tensor_mul(
                                    OT[hp][0:64, sq], oPS[0:64, :],
                                    rb[:, :])
                            else:
                                stB = miscp.tile([64, 512], bf16, tag="st",
                                                 name=f"stB{hp}{n}")
                                nc.vector.tensor_mul(stB[:], oPS[0:64, :],
                                                     rb[:, :])
                                nc.sync.dma_start(OT[hp][64:128, sq], stB[:])

            # ---- output projection (partial over this head-group) ----
            with tc.tile_pool(name="psD", bufs=2, space="PSUM") as psDp, \
                 tc.tile_pool(name="ost", bufs=3) as ostp:
                for m in range(ST):
                    for nn in range(2):
                        ps = psDp.tile([128, 512], f32, tag="d",
                                       name=f"psd{m}{nn}")
                        for k in range(NQ):
                            nc.tensor.matmul(
                                ps[:],
                                lhsT=OT[k][:, m * 128:(m + 1) * 128],
                                rhs=wps[k][:, nn * 512:(nn + 1) * 512],
                                start=(k == 0), stop=(k == NQ - 1))
                        ob = ostp.tile([128, 512], f32, tag="ob",
                                       name=f"ob{m}{nn}")
                        nc.vector.tensor_copy(ob[:], ps[:])
                        nc.sync.dma_start(
                            out[m * 128:(m + 1) * 128,
                                nn * 512:(nn + 1) * 512], ob[:])
    nc.compile()
    return nc


def _get_nc():
    if "nc" not in _CACHE:
        _CACHE["nc"] = _build_bass()
    return _CACHE["nc"]


def _in_maps(x, w_qkv, b_qkv, w_proj, b_proj):
    x = np.asarray(x, np.float32)
    w_qkv = np.asarray(w_qkv, np.float32)
    b_qkv = np.asarray(b_qkv, np.float32)
    w_proj = np.asarray(w_proj, np.float32)
    maps = []
    for c in range(N_CORES):
        b, g = divmod(c, 2)
        cols = slice(g * GC, (g + 1) * GC)
        wqs = w_qkv[:, 0 * DIM:1 * DIM][:, cols]
        wks = w_qkv[:, 1 * DIM:2 * DIM][:, cols]
        wvs = w_qkv[:, 2 * DIM:3 * DIM][:, cols]
        bqs = b_qkv[0 * DIM:1 * DIM][cols]
        bks = b_qkv[1 * DIM:2 * DIM][cols]
        bvs = b_qkv[2 * DIM:3 * DIM][cols]
        rows = slice(g * GC, (g + 1) * GC)
        maps.append({
            "xT": np.ascontiguousarray(x[b].T).astype(BF),
            "wq": wqs.astype(BF),
            "wk": wks.astype(BF),
            "wv": wvs.astype(BF),
            "wp": w_proj[rows, :].astype(BF),
            "bq": np.ascontiguousarray(
                np.broadcast_to(bqs.reshape(4, 128).T[:, :, None],
                                (128, 4, 512)).reshape(128, 2048)),
            "bk": np.ascontiguousarray(
                np.broadcast_to(bks.reshape(4, 128).T[:, :, None],
                                (128, 4, 512)).reshape(128, 2048)),
            "bvb": np.broadcast_to(bvs, (128, GC)).copy(),
        })
    return maps


def kernel(x, w_qkv, b_qkv, w_proj, b_proj, _trace=False):
    from concourse import bass_utils
    nc = _get_nc()
    maps = _in_maps(x, w_qkv, b_qkv, w_proj, b_proj)
    res = bass_utils.run_bass_kernel_spmd(nc, maps,
                                          core_ids=list(range(N_CORES)),
                                          trace=_trace)
    _CACHE["last_result"] = res
    b_proj = np.asarray(b_proj, np.float32)
    outs = np.empty((B, S, DIM), np.float32)
    for b in range(B):
        outs[b] = (res.results[2 * b]["out"] + res.results[2 * b + 1]["out"]
                   + b_proj)
    return outs
